# revision 47
# baseline (speedup 1.0000x reference)
"""Multi-head self-attention Bass kernel for 8 TRN2 NeuronCores.

Problem: B=8, N=1024, C=1024, H=16, D=64, fp32 in/out.
Sharding: data-parallel over batch -- core b computes batch element b
end-to-end; no collectives.

Design (software-pipelined exp stream):
  - all matmuls bf16 (host casts x/w; rel err ~5.6e-3 < 2e-2 gate)
  - prologue: x + the q0/q1 and k0/k1 halves of w0/w2 DMA'd first
    across sync/gpsimd/scalar (w4/w5/w1/w3 avoid the scalar queue --
    DMA issues there would delay the first exps); 30 junk matmuls warm
    the HAM clock; 16 fill units chase the DMA, ordered q0,q1,k0,k1
    (q1 absorbs the wait for w2's tail chunks), then v*blk0
  - stream: 64 slots (pair s, m-tile mi); emission per slot:
        exp_a(g); scores_ta(g+1); fill-chunk; AV(g-8); exp_b(g);
        scores_tb(g+1)
    The scores psum ta/tb [128,1024] each hold BOTH heads of the pair
    in the free dim; the two K=64 scores MMs run concurrently in
    disjoint PE row groups.  Fills ride at 1 chunk (4 MMs) per
    AV-carrying slot (2/slot in pair 0) -- sized so no slot is
    PE-bound.  q/k slabs rotate through 8 buffers (slab s dies after
    pair-s scores)
  - AV unit u=g-8 (4 MMs [65,512]; psum row 64 = softmax denominator
    via a ones column in the v slabs); pair 6 compressed to 2 units/
    slot at g=56-59 so both its divides land in-stream; pair-7 nch0
    accumulates in the fill psum banks (av7imm, g=60-63) and pair-7
    nch1 units ride g=60-62, so only av(7,slot7) remains after the
    last exp
  - divides: mid-stream = psum->SBUF copy, DVE reciprocal, DRAM
    round-trip partition broadcast on gpsimd (latency hidden), DVE
    multiply.  Tail divides (7,0)/(7,1): broadcast the bf16 denominator
    row across partitions 0-63 with a K=1 ones-matmul into a scores
    psum bank, then reciprocal OF THE BROADCAST (satisfies the
    partition-0 DVE constraint) and multiply straight from the acc
    psum -- no DRAM hop, ~2us
  - tail: av(7,7) + junk clock-keeper + both divides overlap, then the
    16 projection groups stream stall-free at full clock; y writes ride
    sync+scalar only (gpsimd drains early in the end-of-kernel cascade)
    with the last two mi groups split into half-width transfers
  - PSUM: scores 2x[128,1024] (4 banks) + AV acc 2x[65,512] (2) +
    fill/proj 2x[128,512] (2) = 8 banks exactly
"""

import os
import sys

sys.path.insert(0, "/opt/trn_rl_repo")

import numpy as np

B, N, C = 8, 1024, 1024
H = 16
D = C // H  # 64
SCALE = D ** -0.5  # 0.125
P = 128
CT = C // P  # 8 contraction tiles of 128

_CACHE = {}

LAST_EXEC_NS = None


def _build():
    import concourse.bacc as bacc
    import concourse.tile as tile
    from concourse import mybir

    fp32 = mybir.dt.float32
    bf16 = mybir.dt.bfloat16
    AFT = mybir.ActivationFunctionType

    nc = bacc.Bacc(
        "TRN2",
        target_bir_lowering=False,
        debug=False,
        enable_asserts=False,
        num_devices=8,
    )
    xT = nc.dram_tensor("xT", [C, N], bf16, kind="ExternalInput")
    wqkvT = nc.dram_tensor("wqkvT", [C, 3 * C], bf16, kind="ExternalInput")
    wprojT = nc.dram_tensor("wprojT", [C, C], bf16, kind="ExternalInput")
    bproj = nc.dram_tensor("bproj", [C], fp32, kind="ExternalInput")
    y = nc.dram_tensor("y", [N, C], fp32, kind="ExternalOutput")

    tap = os.environ.get("MHSA_KERNEL_DEBUG_TAP", "")

    with tile.TileContext(nc) as tc:
        with (
            tc.tile_pool(name="consts", bufs=1) as consts,
            tc.tile_pool(name="xp", bufs=1) as xp,
            tc.tile_pool(name="wq", bufs=4) as wqp,
            tc.tile_pool(name="wp2", bufs=2) as wp2,
            tc.tile_pool(name="qk", bufs=8) as qkp,
            tc.tile_pool(name="vp", bufs=8) as vpp,
            tc.tile_pool(name="et", bufs=26) as etp,
            tc.tile_pool(name="pj", bufs=8) as pjp,
            tc.tile_pool(name="sm", bufs=2) as smp,
            tc.tile_pool(name="avs", bufs=3) as avsp,
            tc.tile_pool(name="rb", bufs=2) as rbp,
            tc.tile_pool(name="tm", bufs=1) as tmp_pool,
            tc.tile_pool(name="ot", bufs=3) as otp,
            tc.tile_pool(name="dscr", bufs=8, space="DRAM") as dscr,
            tc.tile_pool(name="ps_sc", bufs=2, space="PSUM") as ps_sc,
            tc.tile_pool(name="ps_av", bufs=2, space="PSUM") as ps_av,
            tc.tile_pool(name="ps_fl", bufs=2, space="PSUM") as ps_fl,
        ):
            # x resident as ONE tile [128, 8*1024]: column block ci holds
            # xT rows [128ci, 128ci+128)
            xs = xp.tile([P, CT * N], bf16, name="xs", tag="xs")

            def xsl(ci, lo, hi):
                return xs[:, ci * N + lo : ci * N + hi]

            # q/k slabs allocate lazily at fill time and rotate through 8
            # buffers: slab s dies after pair-s scores, well before the
            # slab that reuses its buffer (s+4) is filled.
            qk_tiles = {}

            def qk_tile(kind, s):
                key = (kind, s)
                if key not in qk_tiles:
                    qk_tiles[key] = qkp.tile(
                        [P, N], bf16, name=f"{kind}s{s}", tag="qk"
                    )
                return qk_tiles[key]
            vss = [
                vpp.tile([P, H * 65], bf16, name=f"vs{i}", tag="vs") for i in range(CT)
            ]
            vvs = [v[:].rearrange("p (h e) -> p h e", e=65) for v in vss]
            pjs = [pjp.tile([P, N], bf16, name=f"pj{i}", tag="pj") for i in range(CT)]
            bb = consts.tile([P, C], fp32)

            # ---- ACT table warm-up: tiny exp so the ~2.7us table load
            # happens during the prologue DMA, not at the first real exp.
            junk = smp.tile([1, 16], fp32, name="junk", tag="junk")
            junk2 = smp.tile([1, 16], fp32, name="junk2", tag="junk")
            nc.vector.memset(junk[:], 0.0)
            nc.scalar.activation(junk2[:], junk[:], AFT.Exp, scale=1.0)

            # ---- PE HAM warm-up: dependency-free junk matmuls so the PE
            # clock is at 2.4 GHz (K=8/8) by the time the DMA-fed fills
            # start (the PE drops to 1.2 GHz after any ~3.4us idle gap).
            jw = consts.tile([P, 512], bf16, name="jw")
            nc.vector.memset(jw[:], 0.0)
            # single-partition ones row: K=1 lhsT for the tail divide's
            # denominator-broadcast matmul
            ones1 = consts.tile([1, 64], bf16, name="ones1")
            nc.vector.memset(ones1[:], 1.0)
            jps = ps_fl.tile([1, 512], fp32, name="jfl", tag="fl")

            def emit_junk(n, ps=None):
                # dependency-free matmuls chained on one psum tile; each
                # batch WARs only the previous batch (PE-local), so they
                # run exactly when the PE would otherwise idle, keeping
                # the HAM clock at 2.4 GHz through DMA-gated stretches.
                t = jps if ps is None else ps
                for i in range(n):
                    nc.tensor.matmul(
                        t[:],
                        lhsT=jw[:, 0:1],
                        rhs=jw[:, :],
                        start=(i == 0),
                        stop=(i == n - 1),
                    )

            emit_junk(30)

            # ---- DMAs: plain 2D slab transfers across FOUR queues.
            # Priority: x interleaved with w0 (q0-3) so the q0 fill can
            # chase, then w2 (k0-3), w4/w5 (v), then the rest.
            QS = [nc.sync, nc.gpsimd, nc.scalar]
            NQ = len(QS)
            wtiles = {}

            def walloc():
                return wqp.tile([P, CT * 512], bf16, name="wob", tag="wob")

            def load_w_chunk(wt, oblk, ci, eng, c0=0, c1=512):
                eng.dma_start(
                    wt[:, ci * 512 + c0 : ci * 512 + c1],
                    wqkvT.ap()[
                        ci * P : (ci + 1) * P,
                        oblk * 512 + c0 : oblk * 512 + c1,
                    ],
                )

            def load_w(oblk, flip=0, pool=None, engines=None):
                wt = (pool or wqp).tile([P, CT * 512], bf16, name="wob", tag="wob")
                eng = engines or QS
                for ci in range(CT):
                    load_w_chunk(wt, oblk, ci, eng[(ci + flip) % len(eng)])
                wtiles[oblk] = wt

            def wsl(oblk, ci, lo, hi):
                return wtiles[oblk][:, ci * 512 + lo : ci * 512 + hi]

            # Allocation order w4, w5, w0, w2 so the late w1/w3 loads
            # rotate into the w4/w5 buffers (whose v-fill readers finish
            # first).  DMA issue priority: x + w0 (q0-3) interleaved so
            # the q0 fill can chase, then w2 (k0-3), then w4, w5.
            wtiles[4] = walloc()
            wtiles[5] = walloc()
            wtiles[0] = walloc()
            wtiles[2] = walloc()
            # Critical path: x + the q0/q1 half of w0 + the k0/k1 half
            # of w2 (3 MB instead of 4) interleaved across all 3 queues,
            # so the first four prologue fills start ~7us earlier.  The
            # q2/q3,k2/k3 halves follow w4.  Non-critical blocks avoid
            # the SCALAR queue entirely -- DMA issues on it would delay
            # the first exps (the ACT queue drains in order).
            SG = [nc.sync, nc.gpsimd]
            for ci in range(CT):
                QS[ci % NQ].dma_start(
                    xs[:, ci * N : (ci + 1) * N],
                    xT.ap()[ci * P : (ci + 1) * P, :],
                )
                load_w_chunk(wtiles[0], 0, ci, QS[(ci + 1) % NQ], 0, 256)
                load_w_chunk(wtiles[2], 2, ci, QS[(ci + 2) % NQ], 0, 256)
            for ci in range(CT):
                load_w_chunk(wtiles[4], 4, ci, SG[ci % 2])
            for ci in range(CT):
                load_w_chunk(wtiles[0], 0, ci, QS[ci % NQ], 256, 512)
                load_w_chunk(wtiles[2], 2, ci, QS[(ci + 1) % NQ], 256, 512)
            for ci in range(CT):
                load_w_chunk(wtiles[5], 5, ci, SG[(ci + 1) % 2])
            nc.gpsimd.dma_start(bb[:], bproj.ap().partition_broadcast(P))
            load_w(1, 0, pool=wp2, engines=SG)
            load_w(3, 1, pool=wp2, engines=SG)

            # ---- fill emitters.  A unit is half a q/k slab or half a v
            # slab: 8 MMs + 1 copy.  In the stream they are emitted as
            # two 4-MM chunks so the in-order PE queue never delays the
            # next slot's scores by a full unit.
            def emit_qk_chunk(kind, s, half, chunk, ps_box):
                oblk = (0 if kind == "q" else 2) + s // 4
                dst = qk_tile(kind, s)
                if chunk == 0:
                    ps_box[0] = ps_fl.tile([P, 512], fp32, name="fl", tag="fl")
                ps = ps_box[0]
                for ci in range(4 * chunk, 4 * chunk + 4):
                    nc.tensor.matmul(
                        ps[:],
                        lhsT=wsl(oblk, ci, (s % 4) * P, (s % 4 + 1) * P),
                        rhs=xsl(ci, half * 512, (half + 1) * 512),
                        start=(ci == 0),
                        stop=(ci == CT - 1),
                    )
                if chunk == 1:
                    nc.vector.tensor_copy(
                        dst[:, half * 512 : (half + 1) * 512], ps[:]
                    )

            def emit_v_chunk(mi, vblk, chunk, ps_box):
                if vblk == 0 and chunk == 0:
                    nc.gpsimd.memset(vvs[mi][:, :, 64:65], 1.0)
                if chunk == 0:
                    ps_box[0] = ps_fl.tile([P, 512], fp32, name="fl", tag="fl")
                ps = ps_box[0]
                for ci in range(4 * chunk, 4 * chunk + 4):
                    nc.tensor.matmul(
                        ps[:],
                        lhsT=xsl(ci, mi * P, (mi + 1) * P),
                        rhs=wsl(4 + vblk, ci, 0, 512),
                        start=(ci == 0),
                        stop=(ci == CT - 1),
                    )
                if chunk == 1:
                    nc.vector.tensor_copy(
                        vvs[mi][:, vblk * 8 : (vblk + 1) * 8, 0:64],
                        ps[:].rearrange("p (hh d) -> p hh d", d=64),
                    )

            def emit_unit(u):
                box = [None]
                for chunk in range(2):
                    if u[0] == "v":
                        emit_v_chunk(u[1], u[2], chunk, box)
                    else:
                        emit_qk_chunk(u[0], u[1], u[2], chunk, box)

            # ---- scores MM emitters (2 concurrent K=64 row-group MMs
            # per psum tile; ta covers nch 0, tb covers nch 1; cols
            # 0:512 = even head, 512:1024 = odd head)
            def sc_mms(s, mi, nch):
                t = ps_sc.tile([P, N], fp32, name="sc", tag="sc")
                ks, qs = qk_tile("k", s), qk_tile("q", s)
                for rowlo in (0, 64):
                    nc.tensor.matmul(
                        t[:, (rowlo // 64) * 512 : (rowlo // 64) * 512 + 512],
                        lhsT=ks[rowlo : rowlo + 64, mi * P : (mi + 1) * P],
                        rhs=qs[rowlo : rowlo + 64, nch * 512 : (nch + 1) * 512],
                        start=True,
                        stop=True,
                    )
                return t

            eta = [[None] * CT for _ in range(CT)]
            etb = [[None] * CT for _ in range(CT)]

            def emit_exp(s, mi, nch, t):
                e = etp.tile([P, N], bf16, name="et", tag="et")
                nc.scalar.activation(e[:], t[:], AFT.Exp, scale=SCALE)
                (eta if nch == 0 else etb)[s][mi] = e

            # ---- AV unit (sp, slot): slot//4 = nch, slot%4 = j; 4 MMs
            # (2 mi x 2 rowlo) accumulating [65,512]; row 64 is the
            # softmax denominator via the ones column in the v slabs.
            accs = {}
            tail_divs = {(7, 1)}

            def emit_av_slot(sp, slot, acc_pool=None, do_div=True, half=None):
                # half=0 -> first 2 MMs (mi 2j), half=1 -> second 2 MMs
                # (mi 2j+1) + divide; None -> all 4
                nch = slot // 4
                j = slot % 4
                ets = eta if nch == 0 else etb
                if j == 0 and half != 1:
                    pool = acc_pool if acc_pool is not None else ps_av
                    tg = "fl" if acc_pool is not None else "av"
                    for rowlo in (0, 64):
                        accs[(sp, nch, rowlo)] = pool.tile(
                            [65, 512], fp32, name="av", tag=tg
                        )
                mis = (2 * j, 2 * j + 1)
                if half == 0:
                    mis = (2 * j,)
                elif half == 1:
                    mis = (2 * j + 1,)
                for mi_ in mis:
                    for rowlo in (0, 64):
                        nc.tensor.matmul(
                            accs[(sp, nch, rowlo)][:],
                            lhsT=vvs[mi_][:, 2 * sp + rowlo // 64, :],
                            rhs=ets[sp][mi_][
                                :, (rowlo // 64) * 512 : (rowlo // 64) * 512 + 512
                            ],
                            start=(mi_ == 0),
                            stop=(mi_ == 7),
                        )
                if j == 3 and do_div and half != 0:
                    emit_div(sp, nch, tail=((sp, nch) in tail_divs))

            def emit_av7_imm(mi_):
                # pair-7 nch0 AV accumulated in the filler psum banks (no
                # fills run late in the stream) so its divide lands right
                # at stream end and proj mi0-3 can start immediately.
                if mi_ == 0:
                    for rowlo in (0, 64):
                        accs[(7, 0, rowlo)] = ps_fl.tile(
                            [65, 512], fp32, name="av7", tag="fl"
                        )
                for rowlo in (0, 64):
                    nc.tensor.matmul(
                        accs[(7, 0, rowlo)][:],
                        lhsT=vvs[mi_][:, 14 + rowlo // 64, :],
                        rhs=eta[7][mi_][
                            :, (rowlo // 64) * 512 : (rowlo // 64) * 512 + 512
                        ],
                        start=(mi_ == 0),
                        stop=(mi_ == 7),
                    )

            def emit_div(sp, nch, tail=False):
                # Mid-stream: copy acc psum -> SBUF (releases the bank),
                # DVE reciprocal, DRAM round-trip broadcast on gpsimd
                # (latency hidden), multiply.
                # Tail: broadcast the bf16 denominator row across
                # partitions 0-63 with a K=1 ones-matmul into a scores
                # psum bank (free after the last exp), take the
                # reciprocal OF THE BROADCAST (starts at partition 0,
                # satisfying the DVE constraint), and multiply straight
                # from the acc psum -- no DRAM hop, ~2us total.
                for rowlo in (0, 64):
                    acc = accs.pop((sp, nch, rowlo))
                    dst = pjs[sp][rowlo : rowlo + 64, nch * 512 : (nch + 1) * 512]
                    if tail:
                        dn = smp.tile([1, 512], bf16, name="dn", tag="dn")
                        nc.vector.tensor_copy(dn[:], acc[64:65, :])
                        bc = ps_sc.tile([64, 512], fp32, name="bc", tag="sc")
                        nc.tensor.matmul(
                            bc[:], lhsT=ones1[:], rhs=dn[:], start=True, stop=True
                        )
                        rb = rbp.tile([64, 512], fp32, name="rb", tag="rb")
                        nc.vector.reciprocal_approx_fast(rb[:], bc[:])
                        if rowlo == 0:
                            nc.vector.tensor_mul(dst, acc[0:64, :], rb[:])
                        else:
                            tmp = tmp_pool.tile(
                                [64, 512], bf16, name="tmp", tag="tmp"
                            )
                            nc.vector.tensor_mul(tmp[:], acc[0:64, :], rb[:])
                            nc.sync.dma_start(dst, tmp[:])
                        continue
                    av = avsp.tile([65, 512], fp32, name="avc", tag="avc")
                    nc.vector.tensor_copy(av[:], acc[:])
                    # NB: reciprocal_approx_fast must start at partition 0
                    # and write a separate tile
                    rcp = smp.tile([65, 512], fp32, name="rcp", tag="rcp")
                    nc.vector.reciprocal_approx_fast(rcp[:], av[:])
                    scr = dscr.tile([1, 512], fp32, name="scr", tag="scr")
                    nc.gpsimd.dma_start(scr[:], rcp[64:65, :])
                    rb = rbp.tile([64, 512], fp32, name="rb", tag="rb")
                    nc.gpsimd.dma_start(rb[:], scr[0, :].partition_broadcast(64))
                    if rowlo == 0:
                        nc.vector.tensor_mul(dst, av[0:64, :], rb[:])
                    else:
                        tmp = tmp_pool.tile([64, 512], bf16, name="tmp", tag="tmp")
                        nc.vector.tensor_mul(tmp[:], av[0:64, :], rb[:])
                        nc.gpsimd.dma_start(dst, tmp[:])

            # ---- projection weights (two [128, 8*512] tiles) -- these
            # load LATE into the recycled w4/w5 buffers (v-fill readers
            # are fully emitted by then); w1/w3 live in the wp2 pool and
            # load EARLY so the q4-7/k4-7 fills never wait.
            pwts = []

            def load_wproj():
                for och in range(2):
                    wt = wqp.tile([P, CT * 512], bf16, name="pwt", tag="wob")
                    for ci in range(CT):
                        nc.sync.dma_start(
                            wt[:, ci * 512 : (ci + 1) * 512],
                            wprojT.ap()[
                                ci * P : (ci + 1) * P, och * 512 : (och + 1) * 512
                            ],
                        )
                    pwts.append(wt)

            def emit_proj(mi):
                for och in range(2):
                    ps = ps_fl.tile([P, 512], fp32, name="fl", tag="fl")
                    for ci in range(CT):
                        nc.tensor.matmul(
                            ps[:],
                            lhsT=pjs[ci][:, mi * P : (mi + 1) * P],
                            rhs=pwts[och][:, ci * 512 : (ci + 1) * 512],
                            start=(ci == 0),
                            stop=(ci == CT - 1),
                        )
                    ot = otp.tile([P, 512], fp32, name="ot", tag="ot")
                    nc.vector.tensor_add(
                        ot[:], ps[:], bb[:, och * 512 : (och + 1) * 512]
                    )
                    # y rides sync+scalar only (gpsimd must drain early in
                    # the end-of-kernel cascade); the last two mi groups
                    # split each write across both queues so the final
                    # transfer is half-length.
                    if mi >= 6:
                        nc.sync.dma_start(
                            y.ap()[
                                mi * P : (mi + 1) * P, och * 512 : och * 512 + 256
                            ],
                            ot[:, 0:256],
                        )
                        nc.scalar.dma_start(
                            y.ap()[
                                mi * P : (mi + 1) * P,
                                och * 512 + 256 : (och + 1) * 512,
                            ],
                            ot[:, 256:512],
                        )
                    else:
                        yeng = [nc.sync, nc.scalar][(2 * mi + och) % 2]
                        yeng.dma_start(
                            y.ap()[
                                mi * P : (mi + 1) * P, och * 512 : (och + 1) * 512
                            ],
                            ot[:],
                        )

            run_heads = tap in ("", "pj", "et")
            if run_heads:
                # ---- prologue fills: 16 units, DMA-chased.  q0,k0 gate
                # the stream start; q1,k1 needed by slot 8's lookahead;
                # v*blk0 needed by AV(0) from slot 8.
                # 16-unit prologue: q0,k0 gate the stream start; q1,k1
                # and the v blk0 slabs ride the remaining DMA window.
                # This sizes the in-stream fill load to exactly 1 chunk
                # per AV-carrying slot (no PE-bound stretch).
                # q1 before k0: q1 needs only w0A (already resident when
                # q0 finishes), so it absorbs the wait for w2A's tail
                # chunks instead of the PE idling on k0
                PRO = (
                    [("q", 0, 0), ("q", 0, 1), ("q", 1, 0), ("q", 1, 1)]
                    + [("k", 0, 0), ("k", 0, 1), ("k", 1, 0), ("k", 1, 1)]
                    + [("v", mi, 0) for mi in range(8)]
                )
                for u in PRO:
                    emit_unit(u)

                # ---- in-stream fill chunks, deadline-ordered.
                # q(s),k(s) needed by emission iter 8(s-1)-1 (scores
                # lookahead); v*blk1 by AV(4) at g=40.
                def halves(kind, s):
                    return [(kind, s, 0), (kind, s, 1)]

                stream_units = (
                    [("v", mi, 1) for mi in range(8)]
                    + halves("q", 2) + halves("k", 2)
                    + halves("q", 3) + halves("k", 3)
                    + halves("q", 4) + halves("k", 4)
                    + halves("q", 5) + halves("k", 5)
                    + halves("q", 6) + halves("k", 6)
                    + halves("q", 7) + halves("k", 7)
                )
                # flatten to 4-MM chunks with a shared psum box per unit
                fill_chunks = []
                for u in stream_units:
                    box = [None]
                    for chunk in range(2):
                        fill_chunks.append((u, chunk, box))
                fill_pos = [0]

                def pop_fill(n):
                    for _ in range(n):
                        if fill_pos[0] < len(fill_chunks):
                            u, chunk, box = fill_chunks[fill_pos[0]]
                            fill_pos[0] += 1
                            if u[0] == "v":
                                emit_v_chunk(u[1], u[2], chunk, box)
                            else:
                                emit_qk_chunk(u[0], u[1], u[2], chunk, box)

                # ---- software-pipelined stream.  AV placement: pairs
                # 0-5 lag 8 slots (1 unit/slot, g=8..55); pair 6 is
                # compressed to 2 units/slot over g=56-59 (fills are
                # exhausted by then) so BOTH its divide chains complete
                # in-stream; pair-7 nch0 (av7imm) fills g=60-63.
                av_sched = {
                    56: [(6, 0), (6, 1)],
                    57: [(6, 2), (6, 3)],
                    58: [(6, 4), (6, 5)],
                    59: [(6, 6), (6, 7)],
                    60: [(7, 4)],
                    61: [(7, 5)],
                    62: [(7, 6)],
                }
                imm_sched = {60: [0], 61: [1, 4], 62: [2, 5], 63: [3, 6, 7]}
                ta_cur = sc_mms(0, 0, 0)
                tb_cur = sc_mms(0, 0, 1)
                for g in range(64):
                    s, mi = divmod(g, 8)
                    s1, mi1 = divmod(g + 1, 8)
                    emit_exp(s, mi, 0, ta_cur)
                    # PE queue order: fill + first AV half BEFORE the
                    # next slot's ta MMs -- by the time the PE reaches
                    # ta(g+1), its WAR on exp_a(g) is already satisfied,
                    # so exp_a(g+1) is never late and the stream runs at
                    # the ACT floor.
                    # ta MUST be first in the slot's PE segment: the
                    # exp->PE->exp semaphore round trip (~1.2us) makes
                    # exp_a(g+1) gate on ta(g+1)-done; any MM emitted
                    # before ta adds its latency to every slot (measured
                    # 2596 vs 2413 ns/slot).
                    ta_next = sc_mms(s1, mi1, 0) if g < 63 else None
                    pop_fill(2 if g <= 7 else 1)
                    if 8 <= g <= 55:
                        u = g - 8
                        emit_av_slot(u // 8, u % 8)
                    for sp_slot in av_sched.get(g, ()):
                        emit_av_slot(*sp_slot)
                    for mi_imm in imm_sched.get(g, ()):
                        emit_av7_imm(mi_imm)
                    emit_exp(s, mi, 1, tb_cur)
                    tb_next = sc_mms(s1, mi1, 1) if g < 63 else None
                    ta_cur, tb_cur = ta_next, tb_next
                    if g == 32:
                        load_wproj()
                # drain any leftover fills (shouldn't happen: 64 == 64)
                pop_fill(len(fill_chunks))

                # ---- tail: av(7,7) is the only AV left (the rest ran
                # in-stream); both pair-7 divides use the on-chip
                # MM-broadcast path; a few junk matmuls keep the PE
                # clock at 2.4 GHz through the divide latency, then the
                # projection streams stall-free.
                jt = ps_sc.tile([1, 512], fp32, name="jt", tag="sc")
                emit_av_slot(7, 7, do_div=False)
                emit_junk(6, jt)
                emit_div(7, 0, tail=True)
                emit_div(7, 1, tail=True)
                if tap == "":
                    for mi in range(8):
                        emit_proj(mi)
            else:
                # debug taps for q/k/v only: emit all slabs plainly
                for mi in range(CT):
                    for vblk in range(2):
                        emit_unit(("v", mi, vblk))
                for s in range(CT):
                    for kind in ("q", "k"):
                        for half in range(2):
                            emit_unit((kind, s, half))

            # ---- debug taps
            if tap in ("q", "k"):
                # NB: with 8-buf rotation, slabs 0-3 are overwritten by
                # 4-7 before this dump -- taps q/k only valid for s>=4
                for s in range(CT):
                    ct = otp.tile([P, N], fp32, name="dbgt", tag="dbgt")
                    nc.vector.tensor_copy(ct[:], qk_tile(tap, s)[:])
                    nc.sync.dma_start(y.ap()[s * P : (s + 1) * P, :], ct[:])
            elif tap == "v":
                for mi in range(CT):
                    ct = otp.tile([P, N], fp32, name="dbgt", tag="dbgt")
                    nc.vector.tensor_copy(
                        ct[:].rearrange("p (h d) -> p h d", d=64),
                        vvs[mi][:, :, 0:64],
                    )
                    nc.sync.dma_start(y.ap()[mi * P : (mi + 1) * P, :], ct[:])
            elif tap == "et":
                for mi in range(CT):
                    ct = otp.tile([P, N], fp32, name="dbgt", tag="dbgt")
                    nc.vector.tensor_copy(ct[:], eta[7][mi][:])
                    nc.sync.dma_start(y.ap()[mi * P : (mi + 1) * P, :], ct[:])
            elif tap == "pj":
                for s in range(CT):
                    ct = otp.tile([P, N], fp32, name="dbgt", tag="dbgt")
                    nc.vector.tensor_copy(ct[:], pjs[s][:])
                    nc.sync.dma_start(y.ap()[s * P : (s + 1) * P, :], ct[:])

    nc.compile()
    return nc


def kernel(x, w_qkv, w_proj, b_proj):
    global LAST_EXEC_NS
    import ml_dtypes
    from concourse.bass_utils import run_bass_kernel_spmd

    bf = ml_dtypes.bfloat16
    x = np.asarray(x, dtype=np.float32)
    w_qkv = np.asarray(w_qkv, dtype=np.float32)
    w_proj = np.asarray(w_proj, dtype=np.float32)
    b_proj = np.asarray(b_proj, dtype=np.float32)

    if "nc" not in _CACHE:
        _CACHE["nc"] = _build()
    nc = _CACHE["nc"]

    wqkvT = np.ascontiguousarray(w_qkv.astype(bf).T)
    wprojT = np.ascontiguousarray(w_proj.astype(bf).T)
    xb = x.astype(bf)
    in_maps = [
        {
            "xT": np.ascontiguousarray(xb[b].T),
            "wqkvT": wqkvT,
            "wprojT": wprojT,
            "bproj": b_proj,
        }
        for b in range(B)
    ]
    res = run_bass_kernel_spmd(nc, in_maps, core_ids=list(range(B)))
    if res.exec_time_ns is not None:
        LAST_EXEC_NS = res.exec_time_ns
    return np.stack([res.results[b]["y"] for b in range(B)], axis=0)
